# revision 16
# baseline (speedup 1.0000x reference)
"""Single-head causal attention (B=4, T=2048, C=1024, H=64) on 8 NeuronCores.

Sharding: 8 cores = 4 batches x 2 interleaved halves. Core (b, h) computes
query blocks of 512 rows: h=0 -> rows [0:512] and [1024:1536]; h=1 -> rows
[512:1024] and [1536:2048]. This balances causal work while keeping ONE SPMD
program: all per-core differences enter through input DATA.

Causality, with zero per-chunk instructions:
  - the score matmuls contract over K=66: rows 0:64 are the head dim, rows
    64:65 of the key operand hold per-(block, chunk) biases (0 or -1e30/scale)
    and the query operand holds block-selector rows (1/0). Acausal chunks thus
    come out of the matmul pre-biased to -1e30 and exp() kills them for free.
  - diagonal (partially causal) chunks are masked post-exp by gpsimd
    affine_select (no mask tile, no DMA).

Layout: scores are computed transposed (scoresT[tk, tq]) so softmax sums come
from the PV matmul itself: V is augmented with a ones column -> PV psum row 64
is the denominator.

v3 pipeline (from perfetto evidence): TRN2's PE clock ramps 0.65 -> 1.2 ->
2.4 GHz with 3us of *continuous* execution and any idle gap resets it, so the
whole kernel is laid out as one dense PE stream: garbage-operand warm-up
matmuls spin the PE from t~6us while the first DMAs land; x arrives as 256KB
quarter-chunks in two need-ordered HWDGE streams; projection matmul steps are
injected as PE filler between attention PV matmuls (which are ACT-paced) so
neither engine ever waits long; the block-0 epilogue transposes ride inside
the last attention phase and every epilogue divides + stores per-128-row
slice to shorten the drain. Epilogue and output are bf16.
"""

import numpy as np
import ml_dtypes

import concourse.bass as bass
from concourse import bacc
import concourse.mybir as mybir
import concourse.tile as tile
from concourse.bass_utils import run_bass_kernel_spmd

B, T, C, H = 4, 2048, 1024, 64
P = 128
TQ = 512                 # query block width
NBLK = 2                 # query blocks per core
NQ = NBLK * TQ           # 1024 query rows per core
SCHED = (4, 12)          # full-phase k-chunks per block (compile-time max)
NDIAG = TQ // P          # 4 diagonal chunks per block
KFULL = SCHED[-1] * P    # 1536 k columns needed for full phase
NKCH = KFULL // TQ       # 3 xk column chunks
CCH = C // P             # 8 contraction chunks
NV = NDIAG * NBLK + SCHED[-1]   # 8 diag + 12 full v blocks of 128 rows
SCALE = float(C) ** -0.5
BIGNEG = -1e30 / SCALE   # lands as -1e30 after the exp scale

F32 = mybir.dt.float32
BF16 = mybir.dt.bfloat16
NPBF = ml_dtypes.bfloat16

_CACHE = {}


def build():
    nc = bacc.Bacc()
    xq_d = nc.declare_dram_parameter("xq", [NBLK, P, CCH * TQ], BF16, isOutput=False)
    xk_d = nc.declare_dram_parameter("xk", [NKCH, P, CCH * TQ], BF16, isOutput=False)
    wqv_d = nc.declare_dram_parameter("wqv", [P, CCH * 2 * H], BF16, isOutput=False)
    wk_d = nc.declare_dram_parameter("wk", [P, CCH * H], BF16, isOutput=False)
    wkv_d = nc.declare_dram_parameter("wkv", [P, CCH * 2 * H], BF16, isOutput=False)
    aug_d = nc.declare_dram_parameter("aug", [2, NQ + KFULL + NQ], BF16, isOutput=False)
    st_d = nc.declare_dram_parameter("stair", [P, 896], BF16, isOutput=False)
    idb_d = nc.declare_dram_parameter("identb", [P, P], BF16, isOutput=False)
    out_d = nc.declare_dram_parameter("out", [P, NBLK * NDIAG * H], BF16, isOutput=True)

    EXPF = mybir.ActivationFunctionType.Exp

    with tile.TileContext(nc) as tc:
        with (
            tc.tile_pool(name="big", bufs=1) as big,
            tc.tile_pool(name="work", bufs=6) as work,
            tc.tile_pool(name="epi", bufs=6) as epi,
            tc.tile_pool(name="psp", bufs=2, space="PSUM") as psp,
            tc.tile_pool(name="pss", bufs=2, space="PSUM") as pss,
            tc.tile_pool(name="pspv", bufs=2, space="PSUM") as pspv,
            tc.tile_pool(name="pstr", bufs=2, space="PSUM") as pstr,
        ):
            # ---- DMA triggers, all issued up front. Two HWDGE streams (sync
            # + scalar) advance one need-ordered frontier together; constants
            # ride gpsimd SWDGE.
            identb = big.tile([P, P], BF16)
            nc.sync.dma_start(out=identb[:], in_=idb_d[:])
            wqv = big.tile([P, CCH, 2 * H], BF16)
            nc.sync.dma_start(out=wqv[:], in_=wqv_d[:].rearrange("p (nc h) -> p nc h", nc=CCH))
            wk = big.tile([P, CCH, H], BF16)
            nc.scalar.dma_start(out=wk[:], in_=wk_d[:].rearrange("p (nc h) -> p nc h", nc=CCH))
            # x halves: [P, 4, TQ], one half per HWDGE stream so each tile
            # lands at aggregate rate with 4KB descriptor lines
            xqs = []
            for i in range(NBLK):
                hs = []
                for hh in range(2):
                    t = big.tile([P, 4, TQ], BF16, tag=f"xq{i}h{hh}")
                    eng = [nc.scalar, nc.sync][hh]
                    eng.dma_start(
                        out=t[:],
                        in_=xq_d[i][:, bass.ts(hh, 4 * TQ)].rearrange(
                            "p (c t) -> p c t", c=4))
                    hs.append(t)
                xqs.append(hs)
                if i == 0:
                    # constants needed by the diagonal phases ride the HWDGE
                    # streams right behind xq0
                    stair = big.tile([P, 896], BF16)
                    nc.sync.dma_start(out=stair[:], in_=st_d[:])
                    qb = big.tile([66, NQ], BF16)
                    nc.scalar.dma_start(out=qb[64:66, :], in_=aug_d[:, 0:NQ])
                    kdb = big.tile([66, NQ], BF16)
                    nc.scalar.dma_start(out=kdb[64:66, :], in_=aug_d[:, NQ + KFULL:])
            ktb = big.tile([66, KFULL], BF16)
            nc.scalar.dma_start(out=ktb[64:66, :], in_=aug_d[:, NQ:NQ + KFULL])
            wkv = big.tile([P, CCH, 2 * H], BF16)
            nc.scalar.dma_start(out=wkv[:], in_=wkv_d[:].rearrange("p (nc h) -> p nc h", nc=CCH))
            xks = []
            for i in range(NKCH):
                hs = []
                for hh in range(2):
                    t = big.tile([P, 4, TQ], BF16, tag=f"xk{i}h{hh}")
                    eng = [nc.scalar, nc.sync][hh]
                    eng.dma_start(
                        out=t[:],
                        in_=xk_d[i][:, bass.ts(hh, 4 * TQ)].rearrange(
                            "p (c t) -> p c t", c=4))
                    hs.append(t)
                xks.append(hs)

            # ---- v_aug ones column + PE p-state warm-up on garbage SBUF ----
            vaug = big.tile([P, NV, H + 1], BF16)
            nc.vector.memset(vaug[:, :, H], 1.0)
            wgl = big.tile([P, P], BF16)       # zero operands for PE warm-up
            wgr = big.tile([P, TQ], BF16)
            nc.vector.memset(wgl[:], 0.0)
            nc.vector.memset(wgr[:], 0.0)
            for w in range(8):
                wu = psp.tile([P, TQ], F32, tag="proj")
                nc.tensor.matmul(wu[:], wgl[:], wgr[:], start=True, stop=True)
            for w in range(12):
                wu = pss.tile([P, 256], F32, tag="s")
                nc.tensor.matmul(wu[:], wgl[:], wgr[:, 0:256], start=True, stop=True)

            vdh = big.tile([P, NQ], BF16)      # v of own q rows, partitions 64:128
            vfu = big.tile([P, KFULL], BF16)   # v of prefix rows, partitions 64:128

            # ---- projection steps (closures; used inline or as PE filler) --
            def proj_xq_steps(blk):
                st = {"qv": None, "kd": None}
                sl = bass.ts(blk, TQ)

                def qv_step(hh, jj):
                    def go():
                        if st["qv"] is None:
                            tqv = psp.tile([P, TQ], F32, tag="proj")
                            st["qv"] = tqv
                        for j in range(2):
                            cc = 4 * hh + 2 * jj + j
                            nc.tensor.matmul(st["qv"][:], wqv[:, cc, :],
                                             xqs[blk][hh][:, 2 * jj + j, :],
                                             start=(cc == 0), stop=(cc == CCH - 1))
                        if hh == 1 and jj == 1:
                            o = blk * TQ
                            for j in range(NDIAG):
                                c = bass.ts(j, P)
                                nc.vector.tensor_copy(
                                    vdh[64:128, o + j * P:o + (j + 1) * P],
                                    st["qv"][64:128, c])
                            for hq in range(2):
                                c = bass.ts(hq, 256)
                                nc.vector.tensor_copy(
                                    qb[0:64, o + hq * 256:o + (hq + 1) * 256],
                                    st["qv"][0:64, c])
                    return go

                def kd_step(hh, jj):
                    def go():
                        if st["kd"] is None:
                            tkd = psp.tile([P, TQ], F32, tag="proj")
                            st["kd"] = tkd
                        for j in range(2):
                            cc = 4 * hh + 2 * jj + j
                            nc.tensor.matmul(st["kd"][0:64, :], wk[:, cc, :],
                                             xqs[blk][hh][:, 2 * jj + j, :],
                                             start=(cc == 0), stop=(cc == CCH - 1))
                        if hh == 1 and jj == 1:
                            o = blk * TQ
                            for j in range(NDIAG):
                                c = bass.ts(j, P)
                                nc.vector.tensor_copy(
                                    kdb[0:64, o + j * P:o + (j + 1) * P],
                                    st["kd"][0:64, c])
                    return go

                # kd on half-0 fills the wait for half-1 of xq
                return [qv_step(0, 0), qv_step(0, 1), kd_step(0, 0), kd_step(0, 1),
                        qv_step(1, 0), qv_step(1, 1), kd_step(1, 0), kd_step(1, 1)]

            def proj_xk_steps(seg):
                st = {"kv": None}
                sl = bass.ts(seg, TQ)

                def kv_step(hh, jj):   # two cc per step
                    def go():
                        if st["kv"] is None:
                            tkv = psp.tile([P, TQ], F32, tag="proj")
                            st["kv"] = tkv
                        for j in range(2):
                            cc = 4 * hh + 2 * jj + j
                            nc.tensor.matmul(st["kv"][:], wkv[:, cc, :],
                                             xks[seg][hh][:, 2 * jj + j, :],
                                             start=(cc == 0), stop=(cc == CCH - 1))
                        if hh == 1 and jj == 1:
                            o = seg * TQ
                            for j in range(NDIAG):
                                c = bass.ts(j, P)
                                nc.vector.tensor_copy(
                                    vfu[64:128, o + j * P:o + (j + 1) * P],
                                    st["kv"][64:128, c])
                                nc.vector.tensor_copy(
                                    ktb[0:64, o + j * P:o + (j + 1) * P],
                                    st["kv"][0:64, c])
                    return go

                return [kv_step(hh, jj) for hh in range(2) for jj in range(2)]

            def make_vaug(slot, src_upper, col0):
                tp = pstr.tile([P, H], BF16, tag="tr")
                nc.tensor.transpose(tp[:], src_upper[64:128, col0:col0 + P],
                                    identb[64:128, 64:128])
                nc.vector.tensor_copy(vaug[:, slot, 0:H], tp[:])

            pvs_ps = [None, None]

            def attn_phase(chunks, new_slots, pipe=4, fillers=None):
                """chunks: (blk, kind, c, start, stop). new_slots upfront;
                PVs trail scores by `pipe`; each PV is followed by one filler
                closure (projection work) to keep the PE dense while ACT
                computes the next exp."""
                fillers = list(fillers or [])
                for slot, src, col0 in new_slots:
                    make_vaug(slot, src, col0)
                es = []

                def scores(i):
                    blk, kind, c, _, _ = chunks[i]
                    if kind == "d":
                        slot = blk * NDIAG + c
                        lhsT = kdb[:, blk * TQ + c * P: blk * TQ + (c + 1) * P]
                    else:
                        slot = NBLK * NDIAG + c
                        lhsT = ktb[:, bass.ts(c, P)]
                    s = pss.tile([P, TQ], F32, tag="s")
                    nc.tensor.matmul(s[:], lhsT, qb[0:66, bass.ts(blk, TQ)],
                                     start=True, stop=True)
                    e = work.tile([P, TQ], BF16, tag="e")
                    nc.scalar.activation(e[:], s[:], EXPF, scale=SCALE)
                    if kind == "d":
                        off = 384 - 128 * c
                        nc.vector.tensor_mul(e[:], e[:], stair[:, off:off + TQ])
                    es.append((e, slot))

                def pv(i):
                    blk, kind, c, st_, sp = chunks[i]
                    e, slot = es[i]
                    nc.tensor.matmul(pvs_ps[blk][0:H + 1, :], vaug[:, slot, :],
                                     e[:], start=st_, stop=sp)
                    if fillers:
                        fillers.pop(0)()

                nxt = 0
                for i in range(len(chunks)):
                    scores(i)
                    if i >= pipe - 1:
                        pv(nxt)
                        nxt += 1
                while nxt < len(chunks):
                    pv(nxt)
                    nxt += 1
                for f in fillers:
                    f()

            def epilogue_copies(blk):
                pvs = epi.tile([H + 1, TQ], BF16, tag=f"pvs{blk}")
                for j in range(NDIAG):
                    c = bass.ts(j, P)
                    nc.vector.tensor_copy(pvs[:, c], pvs_ps[blk][0:H + 1, c])
                return pvs

            def epi_tr_step(blk, pvs, j):
                def go():
                    ot = pstr.tile([P, H + 1], BF16, tag="tr")
                    nc.tensor.transpose(ot[:], pvs[:, bass.ts(j, P)],
                                        identb[0:H + 1, 0:H + 1])
                    r = epi.tile([P, 1], F32, tag="r")
                    nc.vector.reciprocal(r[:], ot[:, H:H + 1])
                    ob = bass.ts(blk * NDIAG + j, H)
                    obt = epi.tile([P, H], BF16, tag="ob")
                    nc.vector.tensor_scalar_mul(obt[:], ot[:, 0:H], r[:])
                    nc.sync.dma_start(out=out_d[:, ob], in_=obt[:])
                return go

            # ---- master schedule ----
            diag = lambda blk: [(blk, "d", c, c == 0, False) for c in range(NDIAG)]
            S1 = proj_xq_steps(1)
            S2 = proj_xk_steps(0)
            S3 = proj_xk_steps(1)
            S4 = proj_xk_steps(2)

            for step in proj_xq_steps(0):
                step()
            pv0 = pspv.tile([H + 1, TQ], F32, tag="pv")
            pvs_ps[0] = pv0
            attn_phase(diag(0), [(d, vdh, d * P) for d in range(NDIAG)],
                       fillers=S1[:4])
            for step in S1[4:]:
                step()
            pv1 = pspv.tile([H + 1, TQ], F32, tag="pv")
            pvs_ps[1] = pv1
            attn_phase(diag(1), [(NDIAG + d, vdh, TQ + d * P) for d in range(NDIAG)],
                       fillers=S2)

            phA = ([(0, "f", c, False, c == 3) for c in range(4)] +
                   [(1, "f", c, False, False) for c in range(4)])
            attn_phase(phA, [(NBLK * NDIAG + c, vfu, c * P) for c in range(4)],
                       fillers=S3)
            pvs0 = epilogue_copies(0)

            attn_phase([(1, "f", c, False, False) for c in range(4, 8)],
                       [(NBLK * NDIAG + c, vfu, c * P) for c in range(4, 8)],
                       fillers=S4)

            attn_phase([(1, "f", c, False, c == 11) for c in range(8, 12)],
                       [(NBLK * NDIAG + c, vfu, c * P) for c in range(8, 12)],
                       fillers=[epi_tr_step(0, pvs0, j) for j in range(NDIAG)])

            pvs1 = epilogue_copies(1)
            for j in range(NDIAG):
                epi_tr_step(1, pvs1, j)()
    nc.compile()
    return nc


def _pack_x(xT, cols):
    # xT: [C, T] fp32 -> [P, CCH*W] bf16 in SBUF layout
    a = xT[:, cols]                                   # [C, W]
    a = a.reshape(CCH, P, -1).transpose(1, 0, 2)      # [P, CCH, W]
    return np.ascontiguousarray(a.reshape(P, -1)).astype(NPBF)


def _pack_w(w):
    # w: [C, width] -> [P, CCH*width]
    a = w.reshape(CCH, P, -1).transpose(1, 0, 2)
    return np.ascontiguousarray(a.reshape(P, -1)).astype(NPBF)


def _host_inputs(x, Wk, Wq, Wv):
    wqv = _pack_w(np.concatenate([Wq, Wv], axis=1))
    wkv = _pack_w(np.concatenate([Wk, Wv], axis=1))
    wk = _pack_w(Wk)
    ii = np.arange(P)
    stair = (np.arange(896)[None, :] >= ii[:, None] + 384).astype(NPBF)
    identb = np.eye(P, dtype=NPBF)
    # block-selector rows for qb: row r is 1 on block r's columns
    qaug = np.zeros((2, NQ), np.float32)
    qaug[0, :TQ] = 1.0
    qaug[1, TQ:] = 1.0
    in_maps = []
    for b in range(B):
        xT = np.ascontiguousarray(x[b].T.astype(np.float32))  # [C, T]
        for h in range(2):
            q0s = (0, 1024) if h == 0 else (512, 1536)
            xq = np.stack([_pack_x(xT, slice(q0, q0 + TQ)) for q0 in q0s])
            xk = np.stack([_pack_x(xT, slice(i * TQ, (i + 1) * TQ))
                           for i in range(NKCH)])
            # ktb bias rows: row blk, col t = 0 if chunk t//128 is a (strictly
            # pre-diagonal) causal chunk for this core's block blk, else BIGNEG
            kaug = np.full((2, KFULL), BIGNEG, np.float32)
            for blk, q0 in enumerate(q0s):
                kaug[blk, :q0] = 0.0
            aug = np.concatenate(
                [qaug, kaug, np.zeros((2, NQ), np.float32)], axis=1).astype(NPBF)
            in_maps.append(dict(xq=xq, xk=xk, wqv=wqv, wk=wk, wkv=wkv,
                                aug=aug, stair=stair, identb=identb))
    return in_maps


def kernel(x, Wk, Wq, Wv, trace=False):
    x = np.asarray(x, np.float32)
    in_maps = _host_inputs(x, np.asarray(Wk, np.float32),
                           np.asarray(Wq, np.float32), np.asarray(Wv, np.float32))
    if "nc" not in _CACHE:
        _CACHE["nc"] = build()
    nc = _CACHE["nc"]
    res = run_bass_kernel_spmd(nc, in_maps, list(range(8)), trace=trace)
    out = np.empty((B, T, H), np.float32)
    for b in range(B):
        for h in range(2):
            o = res.results[b * 2 + h]["out"]  # [P, NBLK*NDIAG*H] bf16
            o = np.asarray(o).astype(np.float32).reshape(P, NBLK, NDIAG, H)
            q0s = (0, 1024) if h == 0 else (512, 1536)
            for blk, q0 in enumerate(q0s):
                # row q0 + j*128 + p  <-  o[p, blk, j, :]
                out[b, q0:q0 + TQ] = o[:, blk].transpose(1, 0, 2).reshape(TQ, H)
    kernel.last_exec_time_ns = res.exec_time_ns
    kernel.last_results = res
    return out


# revision 17
# speedup vs baseline: 1.0136x; 1.0136x over previous
"""Single-head causal attention (B=4, T=2048, C=1024, H=64) on 8 NeuronCores.

Sharding: 8 cores = 4 batches x 2 interleaved halves. Core (b, h) computes
query blocks of 512 rows: h=0 -> rows [0:512] and [1024:1536]; h=1 -> rows
[512:1024] and [1536:2048]. This balances causal work while keeping ONE SPMD
program: all per-core differences enter through input DATA.

Causality, with zero per-chunk instructions:
  - the score matmuls contract over K=66: rows 0:64 are the head dim, rows
    64:65 of the key operand hold per-(block, chunk) biases (0 or -1e30/scale)
    and the query operand holds block-selector rows (1/0). Acausal chunks thus
    come out of the matmul pre-biased to -1e30 and exp() kills them for free.
  - diagonal (partially causal) chunks are masked post-exp by gpsimd
    affine_select (no mask tile, no DMA).

Layout: scores are computed transposed (scoresT[tk, tq]) so softmax sums come
from the PV matmul itself: V is augmented with a ones column -> PV psum row 64
is the denominator.

v3 pipeline (from perfetto evidence): TRN2's PE clock ramps 0.65 -> 1.2 ->
2.4 GHz with 3us of *continuous* execution and any idle gap resets it, so the
whole kernel is laid out as one dense PE stream: garbage-operand warm-up
matmuls spin the PE from t~6us while the first DMAs land; x arrives as 256KB
quarter-chunks in two need-ordered HWDGE streams; projection matmul steps are
injected as PE filler between attention PV matmuls (which are ACT-paced) so
neither engine ever waits long; the block-0 epilogue transposes ride inside
the last attention phase and every epilogue divides + stores per-128-row
slice to shorten the drain. Epilogue and output are bf16.
"""

import numpy as np
import ml_dtypes

import concourse.bass as bass
from concourse import bacc
import concourse.mybir as mybir
import concourse.tile as tile
from concourse.bass_utils import run_bass_kernel_spmd

B, T, C, H = 4, 2048, 1024, 64
P = 128
TQ = 512                 # query block width
NBLK = 2                 # query blocks per core
NQ = NBLK * TQ           # 1024 query rows per core
SCHED = (4, 12)          # full-phase k-chunks per block (compile-time max)
NDIAG = TQ // P          # 4 diagonal chunks per block
KFULL = SCHED[-1] * P    # 1536 k columns needed for full phase
NKCH = KFULL // TQ       # 3 xk column chunks
CCH = C // P             # 8 contraction chunks
NV = NDIAG * NBLK + SCHED[-1]   # 8 diag + 12 full v blocks of 128 rows
SCALE = float(C) ** -0.5
BIGNEG = -1e30 / SCALE   # lands as -1e30 after the exp scale

F32 = mybir.dt.float32
BF16 = mybir.dt.bfloat16
NPBF = ml_dtypes.bfloat16

_CACHE = {}


def build():
    nc = bacc.Bacc()
    xq_d = nc.declare_dram_parameter("xq", [NBLK, P, CCH * TQ], BF16, isOutput=False)
    xk_d = nc.declare_dram_parameter("xk", [NKCH, P, CCH * TQ], BF16, isOutput=False)
    wqv_d = nc.declare_dram_parameter("wqv", [P, CCH * 2 * H], BF16, isOutput=False)
    wk_d = nc.declare_dram_parameter("wk", [P, CCH * H], BF16, isOutput=False)
    wkv_d = nc.declare_dram_parameter("wkv", [P, CCH * 2 * H], BF16, isOutput=False)
    aug_d = nc.declare_dram_parameter("aug", [2, NQ + KFULL + NQ], BF16, isOutput=False)
    st_d = nc.declare_dram_parameter("stair", [P, 896], BF16, isOutput=False)
    idb_d = nc.declare_dram_parameter("identb", [P, P], BF16, isOutput=False)
    out_d = nc.declare_dram_parameter("out", [P, NBLK * NDIAG * H], BF16, isOutput=True)

    EXPF = mybir.ActivationFunctionType.Exp

    with tile.TileContext(nc) as tc:
        with (
            tc.tile_pool(name="big", bufs=1) as big,
            tc.tile_pool(name="work", bufs=6) as work,
            tc.tile_pool(name="epi", bufs=6) as epi,
            tc.tile_pool(name="psp", bufs=2, space="PSUM") as psp,
            tc.tile_pool(name="pss", bufs=2, space="PSUM") as pss,
            tc.tile_pool(name="pspv", bufs=2, space="PSUM") as pspv,
            tc.tile_pool(name="pstr", bufs=2, space="PSUM") as pstr,
        ):
            # ---- DMA triggers, all issued up front. Two HWDGE streams (sync
            # + scalar) advance one need-ordered frontier together; constants
            # ride gpsimd SWDGE.
            stair = big.tile([P, 896], BF16)
            nc.sync.dma_start(out=stair[:], in_=st_d[:])
            wqv = big.tile([P, CCH, 2 * H], BF16)
            nc.sync.dma_start(out=wqv[:], in_=wqv_d[:].rearrange("p (nc h) -> p nc h", nc=CCH))
            qb = big.tile([66, NQ], BF16)
            nc.scalar.dma_start(out=qb[64:66, :], in_=aug_d[:, 0:NQ])
            kdb = big.tile([66, NQ], BF16)
            nc.scalar.dma_start(out=kdb[64:66, :], in_=aug_d[:, NQ + KFULL:])
            ktb = big.tile([66, KFULL], BF16)
            nc.scalar.dma_start(out=ktb[64:66, :], in_=aug_d[:, NQ:NQ + KFULL])
            wk = big.tile([P, CCH, H], BF16)
            nc.scalar.dma_start(out=wk[:], in_=wk_d[:].rearrange("p (nc h) -> p nc h", nc=CCH))
            identb = big.tile([P, P], BF16)
            nc.scalar.dma_start(out=identb[:], in_=idb_d[:])
            # x halves: [P, 4, TQ], one half per HWDGE stream so each tile
            # lands at aggregate rate with 4KB descriptor lines
            xqs = []
            for i in range(NBLK):
                hs = []
                for hh in range(2):
                    t = big.tile([P, 4, TQ], BF16, tag=f"xq{i}h{hh}")
                    eng = [nc.scalar, nc.sync][hh]
                    eng.dma_start(
                        out=t[:],
                        in_=xq_d[i][:, bass.ts(hh, 4 * TQ)].rearrange(
                            "p (c t) -> p c t", c=4))
                    hs.append(t)
                xqs.append(hs)
            wkv = big.tile([P, CCH, 2 * H], BF16)
            nc.scalar.dma_start(out=wkv[:], in_=wkv_d[:].rearrange("p (nc h) -> p nc h", nc=CCH))
            xks = []
            for i in range(NKCH):
                hs = []
                for hh in range(2):
                    t = big.tile([P, 4, TQ], BF16, tag=f"xk{i}h{hh}")
                    eng = [nc.scalar, nc.sync][hh]
                    eng.dma_start(
                        out=t[:],
                        in_=xk_d[i][:, bass.ts(hh, 4 * TQ)].rearrange(
                            "p (c t) -> p c t", c=4))
                    hs.append(t)
                xks.append(hs)

            # ---- v_aug ones column + PE p-state warm-up on garbage SBUF ----
            vaug = big.tile([P, NV, H + 1], BF16)
            nc.vector.memset(vaug[:, :, H], 1.0)
            wgl = big.tile([P, P], BF16)       # zero operands for PE warm-up
            wgr = big.tile([P, TQ], BF16)
            nc.vector.memset(wgl[:], 0.0)
            nc.vector.memset(wgr[:], 0.0)
            for w in range(8):
                wu = psp.tile([P, TQ], F32, tag="proj")
                nc.tensor.matmul(wu[:], wgl[:], wgr[:], start=True, stop=True)
            for w in range(12):
                wu = pss.tile([P, 256], F32, tag="s")
                nc.tensor.matmul(wu[:], wgl[:], wgr[:, 0:256], start=True, stop=True)

            vdh = big.tile([P, NQ], BF16)      # v of own q rows, partitions 64:128
            vfu = big.tile([P, KFULL], BF16)   # v of prefix rows, partitions 64:128

            # ---- projection steps (closures; used inline or as PE filler) --
            def proj_xq_steps(blk):
                st = {"qv": None, "kd": None}
                sl = bass.ts(blk, TQ)

                def qv_step(hh, jj):
                    def go():
                        if st["qv"] is None:
                            tqv = psp.tile([P, TQ], F32, tag="proj")
                            st["qv"] = tqv
                        for j in range(2):
                            cc = 4 * hh + 2 * jj + j
                            nc.tensor.matmul(st["qv"][:], wqv[:, cc, :],
                                             xqs[blk][hh][:, 2 * jj + j, :],
                                             start=(cc == 0), stop=(cc == CCH - 1))
                        if hh == 1 and jj == 1:
                            o = blk * TQ
                            for j in range(NDIAG):
                                c = bass.ts(j, P)
                                nc.vector.tensor_copy(
                                    vdh[64:128, o + j * P:o + (j + 1) * P],
                                    st["qv"][64:128, c])
                            for hq in range(2):
                                c = bass.ts(hq, 256)
                                nc.vector.tensor_copy(
                                    qb[0:64, o + hq * 256:o + (hq + 1) * 256],
                                    st["qv"][0:64, c])
                    return go

                def kd_step(hh, jj):
                    def go():
                        if st["kd"] is None:
                            tkd = psp.tile([P, TQ], F32, tag="proj")
                            st["kd"] = tkd
                        for j in range(2):
                            cc = 4 * hh + 2 * jj + j
                            nc.tensor.matmul(st["kd"][0:64, :], wk[:, cc, :],
                                             xqs[blk][hh][:, 2 * jj + j, :],
                                             start=(cc == 0), stop=(cc == CCH - 1))
                        if hh == 1 and jj == 1:
                            o = blk * TQ
                            for j in range(NDIAG):
                                c = bass.ts(j, P)
                                nc.vector.tensor_copy(
                                    kdb[0:64, o + j * P:o + (j + 1) * P],
                                    st["kd"][0:64, c])
                    return go

                # kd on half-0 fills the wait for half-1 of xq
                return [qv_step(0, 0), qv_step(0, 1), kd_step(0, 0), kd_step(0, 1),
                        qv_step(1, 0), qv_step(1, 1), kd_step(1, 0), kd_step(1, 1)]

            def proj_xk_steps(seg):
                st = {"kv": None}
                sl = bass.ts(seg, TQ)

                def kv_step(hh, jj):   # two cc per step
                    def go():
                        if st["kv"] is None:
                            tkv = psp.tile([P, TQ], F32, tag="proj")
                            st["kv"] = tkv
                        for j in range(2):
                            cc = 4 * hh + 2 * jj + j
                            nc.tensor.matmul(st["kv"][:], wkv[:, cc, :],
                                             xks[seg][hh][:, 2 * jj + j, :],
                                             start=(cc == 0), stop=(cc == CCH - 1))
                        if hh == 1 and jj == 1:
                            o = seg * TQ
                            for j in range(NDIAG):
                                c = bass.ts(j, P)
                                nc.vector.tensor_copy(
                                    vfu[64:128, o + j * P:o + (j + 1) * P],
                                    st["kv"][64:128, c])
                                nc.vector.tensor_copy(
                                    ktb[0:64, o + j * P:o + (j + 1) * P],
                                    st["kv"][0:64, c])
                    return go

                return [kv_step(hh, jj) for hh in range(2) for jj in range(2)]

            def make_vaug(slot, src_upper, col0):
                tp = pstr.tile([P, H], BF16, tag="tr")
                nc.tensor.transpose(tp[:], src_upper[64:128, col0:col0 + P],
                                    identb[64:128, 64:128])
                nc.vector.tensor_copy(vaug[:, slot, 0:H], tp[:])

            pvs_ps = [None, None]

            def attn_phase(chunks, new_slots, pipe=4, fillers=None):
                """chunks: (blk, kind, c, start, stop). new_slots upfront;
                PVs trail scores by `pipe`; each PV is followed by one filler
                closure (projection work) to keep the PE dense while ACT
                computes the next exp."""
                fillers = list(fillers or [])
                for slot, src, col0 in new_slots:
                    make_vaug(slot, src, col0)
                es = []

                def scores(i):
                    blk, kind, c, _, _ = chunks[i]
                    if kind == "d":
                        slot = blk * NDIAG + c
                        lhsT = kdb[:, blk * TQ + c * P: blk * TQ + (c + 1) * P]
                    else:
                        slot = NBLK * NDIAG + c
                        lhsT = ktb[:, bass.ts(c, P)]
                    s = pss.tile([P, TQ], F32, tag="s")
                    nc.tensor.matmul(s[:], lhsT, qb[0:66, bass.ts(blk, TQ)],
                                     start=True, stop=True)
                    e = work.tile([P, TQ], BF16, tag="e")
                    nc.scalar.activation(e[:], s[:], EXPF, scale=SCALE)
                    if kind == "d":
                        off = 384 - 128 * c
                        nc.vector.tensor_mul(e[:], e[:], stair[:, off:off + TQ])
                    es.append((e, slot))

                def pv(i):
                    blk, kind, c, st_, sp = chunks[i]
                    e, slot = es[i]
                    nc.tensor.matmul(pvs_ps[blk][0:H + 1, :], vaug[:, slot, :],
                                     e[:], start=st_, stop=sp)
                    if fillers:
                        fillers.pop(0)()

                nxt = 0
                for i in range(len(chunks)):
                    scores(i)
                    if i >= pipe - 1:
                        pv(nxt)
                        nxt += 1
                while nxt < len(chunks):
                    pv(nxt)
                    nxt += 1
                for f in fillers:
                    f()

            def epilogue_copies(blk):
                pvs = epi.tile([H + 1, TQ], BF16, tag=f"pvs{blk}")
                for j in range(NDIAG):
                    c = bass.ts(j, P)
                    nc.vector.tensor_copy(pvs[:, c], pvs_ps[blk][0:H + 1, c])
                return pvs

            def epi_tr_step(blk, pvs, j):
                def go():
                    ot = pstr.tile([P, H + 1], BF16, tag="tr")
                    nc.tensor.transpose(ot[:], pvs[:, bass.ts(j, P)],
                                        identb[0:H + 1, 0:H + 1])
                    r = epi.tile([P, 1], F32, tag="r")
                    nc.vector.reciprocal(r[:], ot[:, H:H + 1])
                    ob = bass.ts(blk * NDIAG + j, H)
                    obt = epi.tile([P, H], BF16, tag="ob")
                    nc.vector.tensor_scalar_mul(obt[:], ot[:, 0:H], r[:])
                    nc.sync.dma_start(out=out_d[:, ob], in_=obt[:])
                return go

            # ---- master schedule ----
            diag = lambda blk: [(blk, "d", c, c == 0, False) for c in range(NDIAG)]
            S1 = proj_xq_steps(1)
            S2 = proj_xk_steps(0)
            S3 = proj_xk_steps(1)
            S4 = proj_xk_steps(2)

            for step in proj_xq_steps(0):
                step()
            pv0 = pspv.tile([H + 1, TQ], F32, tag="pv")
            pvs_ps[0] = pv0
            attn_phase(diag(0), [(d, vdh, d * P) for d in range(NDIAG)],
                       fillers=S1[:4])
            for step in S1[4:]:
                step()
            pv1 = pspv.tile([H + 1, TQ], F32, tag="pv")
            pvs_ps[1] = pv1
            attn_phase(diag(1), [(NDIAG + d, vdh, TQ + d * P) for d in range(NDIAG)],
                       fillers=S2)

            phA = ([(0, "f", c, False, c == 3) for c in range(4)] +
                   [(1, "f", c, False, False) for c in range(4)])
            attn_phase(phA, [(NBLK * NDIAG + c, vfu, c * P) for c in range(4)],
                       fillers=S3)
            pvs0 = epilogue_copies(0)

            attn_phase([(1, "f", c, False, False) for c in range(4, 8)],
                       [(NBLK * NDIAG + c, vfu, c * P) for c in range(4, 8)],
                       fillers=S4)

            attn_phase([(1, "f", c, False, c == 11) for c in range(8, 12)],
                       [(NBLK * NDIAG + c, vfu, c * P) for c in range(8, 12)],
                       fillers=[epi_tr_step(0, pvs0, j) for j in range(NDIAG)])

            pvs1 = epilogue_copies(1)
            for j in range(NDIAG):
                epi_tr_step(1, pvs1, j)()
    nc.compile()
    return nc


def _pack_x(xT, cols):
    # xT: [C, T] fp32 -> [P, CCH*W] bf16 in SBUF layout
    a = xT[:, cols]                                   # [C, W]
    a = a.reshape(CCH, P, -1).transpose(1, 0, 2)      # [P, CCH, W]
    return np.ascontiguousarray(a.reshape(P, -1)).astype(NPBF)


def _pack_w(w):
    # w: [C, width] -> [P, CCH*width]
    a = w.reshape(CCH, P, -1).transpose(1, 0, 2)
    return np.ascontiguousarray(a.reshape(P, -1)).astype(NPBF)


def _host_inputs(x, Wk, Wq, Wv):
    wqv = _pack_w(np.concatenate([Wq, Wv], axis=1))
    wkv = _pack_w(np.concatenate([Wk, Wv], axis=1))
    wk = _pack_w(Wk)
    ii = np.arange(P)
    stair = (np.arange(896)[None, :] >= ii[:, None] + 384).astype(NPBF)
    identb = np.eye(P, dtype=NPBF)
    # block-selector rows for qb: row r is 1 on block r's columns
    qaug = np.zeros((2, NQ), np.float32)
    qaug[0, :TQ] = 1.0
    qaug[1, TQ:] = 1.0
    in_maps = []
    for b in range(B):
        xT = np.ascontiguousarray(x[b].T.astype(np.float32))  # [C, T]
        for h in range(2):
            q0s = (0, 1024) if h == 0 else (512, 1536)
            xq = np.stack([_pack_x(xT, slice(q0, q0 + TQ)) for q0 in q0s])
            xk = np.stack([_pack_x(xT, slice(i * TQ, (i + 1) * TQ))
                           for i in range(NKCH)])
            # ktb bias rows: row blk, col t = 0 if chunk t//128 is a (strictly
            # pre-diagonal) causal chunk for this core's block blk, else BIGNEG
            kaug = np.full((2, KFULL), BIGNEG, np.float32)
            for blk, q0 in enumerate(q0s):
                kaug[blk, :q0] = 0.0
            aug = np.concatenate(
                [qaug, kaug, np.zeros((2, NQ), np.float32)], axis=1).astype(NPBF)
            in_maps.append(dict(xq=xq, xk=xk, wqv=wqv, wk=wk, wkv=wkv,
                                aug=aug, stair=stair, identb=identb))
    return in_maps


def kernel(x, Wk, Wq, Wv, trace=False):
    x = np.asarray(x, np.float32)
    in_maps = _host_inputs(x, np.asarray(Wk, np.float32),
                           np.asarray(Wq, np.float32), np.asarray(Wv, np.float32))
    if "nc" not in _CACHE:
        _CACHE["nc"] = build()
    nc = _CACHE["nc"]
    res = run_bass_kernel_spmd(nc, in_maps, list(range(8)), trace=trace)
    out = np.empty((B, T, H), np.float32)
    for b in range(B):
        for h in range(2):
            o = res.results[b * 2 + h]["out"]  # [P, NBLK*NDIAG*H] bf16
            o = np.asarray(o).astype(np.float32).reshape(P, NBLK, NDIAG, H)
            q0s = (0, 1024) if h == 0 else (512, 1536)
            for blk, q0 in enumerate(q0s):
                # row q0 + j*128 + p  <-  o[p, blk, j, :]
                out[b, q0:q0 + TQ] = o[:, blk].transpose(1, 0, 2).reshape(TQ, H)
    kernel.last_exec_time_ns = res.exec_time_ns
    kernel.last_results = res
    return out


# revision 19
# speedup vs baseline: 1.0766x; 1.0621x over previous
"""Single-head causal attention (B=4, T=2048, C=1024, H=64) on 8 NeuronCores.

Sharding: 8 cores = 4 batches x 2 interleaved halves. Core (b, h) computes
query blocks of 512 rows: h=0 -> rows [0:512] and [1024:1536]; h=1 -> rows
[512:1024] and [1536:2048]. This balances causal work while keeping ONE SPMD
program: all per-core differences enter through input DATA.

Causality, with zero per-chunk instructions:
  - the score matmuls contract over K=66: rows 0:64 are the head dim, rows
    64:65 of the key operand hold per-(block, chunk) biases (0 or -1e30/scale)
    and the query operand holds block-selector rows (1/0). Acausal chunks thus
    come out of the matmul pre-biased to -1e30 and exp() kills them for free.
  - diagonal (partially causal) chunks are masked post-exp by gpsimd
    affine_select (no mask tile, no DMA).

Layout: scores are computed transposed (scoresT[tk, tq]) so softmax sums come
from the PV matmul itself: V is augmented with a ones column -> PV psum row 64
is the denominator.

v3 pipeline (from perfetto evidence): TRN2's PE clock ramps 0.65 -> 1.2 ->
2.4 GHz with 3us of *continuous* execution and any idle gap resets it, so the
whole kernel is laid out as one dense PE stream: garbage-operand warm-up
matmuls spin the PE from t~6us while the first DMAs land; x arrives as 256KB
quarter-chunks in two need-ordered HWDGE streams; projection matmul steps are
injected as PE filler between attention PV matmuls (which are ACT-paced) so
neither engine ever waits long; the block-0 epilogue transposes ride inside
the last attention phase and every epilogue divides + stores per-128-row
slice to shorten the drain. Epilogue and output are bf16.
"""

import numpy as np
import ml_dtypes

import concourse.bass as bass
from concourse import bacc
import concourse.mybir as mybir
import concourse.tile as tile
from concourse.bass_utils import run_bass_kernel_spmd

B, T, C, H = 4, 2048, 1024, 64
P = 128
TQ = 512                 # query block width
NBLK = 2                 # query blocks per core
NQ = NBLK * TQ           # 1024 query rows per core
SCHED = (4, 12)          # full-phase k-chunks per block (compile-time max)
NDIAG = TQ // P          # 4 diagonal chunks per block
KFULL = SCHED[-1] * P    # 1536 k columns needed for full phase
NKCH = KFULL // TQ       # 3 xk column chunks
CCH = C // P             # 8 contraction chunks
NV = NDIAG * NBLK + SCHED[-1]   # 8 diag + 12 full v blocks of 128 rows
SCALE = float(C) ** -0.5
BIGNEG = -1e30 / SCALE   # lands as -1e30 after the exp scale

F32 = mybir.dt.float32
BF16 = mybir.dt.bfloat16
NPBF = ml_dtypes.bfloat16

_CACHE = {}


def build():
    nc = bacc.Bacc()
    xq_d = nc.declare_dram_parameter("xq", [NBLK, P, CCH * TQ], BF16, isOutput=False)
    xk_d = nc.declare_dram_parameter("xk", [NKCH, P, CCH * TQ], BF16, isOutput=False)
    wqv_d = nc.declare_dram_parameter("wqv", [P, CCH * 2 * H], BF16, isOutput=False)
    wk_d = nc.declare_dram_parameter("wk", [P, CCH * H], BF16, isOutput=False)
    wkv_d = nc.declare_dram_parameter("wkv", [P, CCH * 2 * H], BF16, isOutput=False)
    aug_d = nc.declare_dram_parameter("aug", [2, KFULL], BF16, isOutput=False)
    st_d = nc.declare_dram_parameter("stair", [P, 896], BF16, isOutput=False)
    idb_d = nc.declare_dram_parameter("identb", [P, P], BF16, isOutput=False)
    out_d = nc.declare_dram_parameter("out", [P, NBLK * NDIAG * H], BF16, isOutput=True)

    EXPF = mybir.ActivationFunctionType.Exp

    with tile.TileContext(nc) as tc:
        with (
            tc.tile_pool(name="big", bufs=1) as big,
            tc.tile_pool(name="work", bufs=6) as work,
            tc.tile_pool(name="epi", bufs=6) as epi,
            tc.tile_pool(name="psp", bufs=2, space="PSUM") as psp,
            tc.tile_pool(name="pss", bufs=2, space="PSUM") as pss,
            tc.tile_pool(name="pspv", bufs=2, space="PSUM") as pspv,
            tc.tile_pool(name="pstr", bufs=2, space="PSUM") as pstr,
        ):
            # ---- DMA triggers, all issued up front. Two HWDGE streams (sync
            # + scalar) advance one need-ordered frontier together; constants
            # ride gpsimd SWDGE.
            # per-core ktb bias rows: tiny SWDGE transfer, fired first
            ktb = big.tile([66, KFULL], BF16)
            nc.gpsimd.dma_start(out=ktb[64:66, :], in_=aug_d[:])
            stair = big.tile([P, 896], BF16)
            nc.sync.dma_start(out=stair[:], in_=st_d[:])
            wqv = big.tile([P, CCH, 2 * H], BF16)
            nc.sync.dma_start(out=wqv[:], in_=wqv_d[:].rearrange("p (nc h) -> p nc h", nc=CCH))
            wk = big.tile([P, CCH, H], BF16)
            nc.scalar.dma_start(out=wk[:], in_=wk_d[:].rearrange("p (nc h) -> p nc h", nc=CCH))
            identb = big.tile([P, P], BF16)
            nc.scalar.dma_start(out=identb[:], in_=idb_d[:])
            # core-uniform aug rows are generated on device:
            # qb rows 64:65 select block 0/1; kdb bias rows are zero
            qb = big.tile([66, NQ], BF16)
            nc.vector.memset(qb[64:66, :], 0.0)
            nc.vector.memset(qb[64:66, TQ:NQ], 1.0)
            nc.vector.memset(qb[64:65, TQ:NQ], 0.0)
            nc.vector.memset(qb[64:65, 0:TQ], 1.0)
            kdb = big.tile([66, NQ], BF16)
            nc.vector.memset(kdb[64:66, :], 0.0)
            # x halves: [P, 4, TQ], one half per HWDGE stream so each tile
            # lands at aggregate rate with 4KB descriptor lines
            xqs = []
            for i in range(NBLK):
                hs = []
                for hh in range(2):
                    t = big.tile([P, 4, TQ], BF16, tag=f"xq{i}h{hh}")
                    eng = [nc.scalar, nc.sync][hh]
                    eng.dma_start(
                        out=t[:],
                        in_=xq_d[i][:, bass.ts(hh, 4 * TQ)].rearrange(
                            "p (c t) -> p c t", c=4))
                    hs.append(t)
                xqs.append(hs)
            wkv = big.tile([P, CCH, 2 * H], BF16)
            nc.scalar.dma_start(out=wkv[:], in_=wkv_d[:].rearrange("p (nc h) -> p nc h", nc=CCH))
            xks = []
            for i in range(NKCH):
                hs = []
                for hh in range(2):
                    t = big.tile([P, 4, TQ], BF16, tag=f"xk{i}h{hh}")
                    eng = [nc.scalar, nc.sync][hh]
                    eng.dma_start(
                        out=t[:],
                        in_=xk_d[i][:, bass.ts(hh, 4 * TQ)].rearrange(
                            "p (c t) -> p c t", c=4))
                    hs.append(t)
                xks.append(hs)

            # ---- v_aug ones column + PE p-state warm-up on garbage SBUF ----
            vaug = big.tile([P, NV, H + 1], BF16)
            nc.vector.memset(vaug[:, :, H], 1.0)
            wgl = big.tile([P, P], BF16)       # zero operands for PE warm-up
            wgr = big.tile([P, TQ], BF16)
            nc.vector.memset(wgl[:], 0.0)
            nc.vector.memset(wgr[:], 0.0)
            for w in range(8):
                wu = psp.tile([P, TQ], F32, tag="proj")
                nc.tensor.matmul(wu[:], wgl[:], wgr[:], start=True, stop=True)
            for w in range(12):
                wu = pss.tile([P, 256], F32, tag="s")
                nc.tensor.matmul(wu[:], wgl[:], wgr[:, 0:256], start=True, stop=True)

            vdh = big.tile([P, NQ], BF16)      # v of own q rows, partitions 64:128
            vfu = big.tile([P, KFULL], BF16)   # v of prefix rows, partitions 64:128

            # ---- projection steps (closures; used inline or as PE filler) --
            def proj_xq_steps(blk):
                st = {"qv": None, "kd": None}
                sl = bass.ts(blk, TQ)

                def qv_step(hh, jj):
                    def go():
                        if st["qv"] is None:
                            tqv = psp.tile([P, TQ], F32, tag="proj")
                            st["qv"] = tqv
                        for j in range(2):
                            cc = 4 * hh + 2 * jj + j
                            nc.tensor.matmul(st["qv"][:], wqv[:, cc, :],
                                             xqs[blk][hh][:, 2 * jj + j, :],
                                             start=(cc == 0), stop=(cc == CCH - 1))
                        if hh == 1 and jj == 1:
                            o = blk * TQ
                            for j in range(NDIAG):
                                c = bass.ts(j, P)
                                nc.vector.tensor_copy(
                                    vdh[64:128, o + j * P:o + (j + 1) * P],
                                    st["qv"][64:128, c])
                            for hq in range(2):
                                c = bass.ts(hq, 256)
                                nc.vector.tensor_copy(
                                    qb[0:64, o + hq * 256:o + (hq + 1) * 256],
                                    st["qv"][0:64, c])
                    return go

                def kd_step(hh, jj):
                    def go():
                        if st["kd"] is None:
                            tkd = psp.tile([P, TQ], F32, tag="proj")
                            st["kd"] = tkd
                        for j in range(2):
                            cc = 4 * hh + 2 * jj + j
                            nc.tensor.matmul(st["kd"][0:64, :], wk[:, cc, :],
                                             xqs[blk][hh][:, 2 * jj + j, :],
                                             start=(cc == 0), stop=(cc == CCH - 1))
                        if hh == 1 and jj == 1:
                            o = blk * TQ
                            for j in range(NDIAG):
                                c = bass.ts(j, P)
                                nc.vector.tensor_copy(
                                    kdb[0:64, o + j * P:o + (j + 1) * P],
                                    st["kd"][0:64, c])
                    return go

                # kd on half-0 fills the wait for half-1 of xq
                return [qv_step(0, 0), qv_step(0, 1), kd_step(0, 0), kd_step(0, 1),
                        qv_step(1, 0), qv_step(1, 1), kd_step(1, 0), kd_step(1, 1)]

            def proj_xk_steps(seg):
                st = {"kv": None}
                sl = bass.ts(seg, TQ)

                def kv_step(hh, jj):   # two cc per step
                    def go():
                        if st["kv"] is None:
                            tkv = psp.tile([P, TQ], F32, tag="proj")
                            st["kv"] = tkv
                        for j in range(2):
                            cc = 4 * hh + 2 * jj + j
                            nc.tensor.matmul(st["kv"][:], wkv[:, cc, :],
                                             xks[seg][hh][:, 2 * jj + j, :],
                                             start=(cc == 0), stop=(cc == CCH - 1))
                        if hh == 1 and jj == 1:
                            o = seg * TQ
                            for j in range(NDIAG):
                                c = bass.ts(j, P)
                                nc.vector.tensor_copy(
                                    vfu[64:128, o + j * P:o + (j + 1) * P],
                                    st["kv"][64:128, c])
                                nc.vector.tensor_copy(
                                    ktb[0:64, o + j * P:o + (j + 1) * P],
                                    st["kv"][0:64, c])
                    return go

                return [kv_step(hh, jj) for hh in range(2) for jj in range(2)]

            def make_vaug(slot, src_upper, col0):
                tp = pstr.tile([P, H], BF16, tag="tr")
                nc.tensor.transpose(tp[:], src_upper[64:128, col0:col0 + P],
                                    identb[64:128, 64:128])
                nc.vector.tensor_copy(vaug[:, slot, 0:H], tp[:])

            pvs_ps = [None, None]

            def attn_phase(chunks, new_slots, pipe=4, fillers=None):
                """chunks: (blk, kind, c, start, stop). new_slots upfront;
                PVs trail scores by `pipe`; each PV is followed by one filler
                closure (projection work) to keep the PE dense while ACT
                computes the next exp."""
                fillers = list(fillers or [])
                for slot, src, col0 in new_slots:
                    make_vaug(slot, src, col0)
                es = []

                def scores(i):
                    blk, kind, c, _, _ = chunks[i]
                    if kind == "d":
                        slot = blk * NDIAG + c
                        lhsT = kdb[:, blk * TQ + c * P: blk * TQ + (c + 1) * P]
                    else:
                        slot = NBLK * NDIAG + c
                        lhsT = ktb[:, bass.ts(c, P)]
                    s = pss.tile([P, TQ], F32, tag="s")
                    nc.tensor.matmul(s[:], lhsT, qb[0:66, bass.ts(blk, TQ)],
                                     start=True, stop=True)
                    e = work.tile([P, TQ], BF16, tag="e")
                    nc.scalar.activation(e[:], s[:], EXPF, scale=SCALE)
                    if kind == "d":
                        off = 384 - 128 * c
                        nc.vector.tensor_mul(e[:], e[:], stair[:, off:off + TQ])
                    es.append((e, slot))

                def pv(i):
                    blk, kind, c, st_, sp = chunks[i]
                    e, slot = es[i]
                    nc.tensor.matmul(pvs_ps[blk][0:H + 1, :], vaug[:, slot, :],
                                     e[:], start=st_, stop=sp)
                    if fillers:
                        fillers.pop(0)()

                nxt = 0
                for i in range(len(chunks)):
                    scores(i)
                    if i >= pipe - 1:
                        pv(nxt)
                        nxt += 1
                while nxt < len(chunks):
                    pv(nxt)
                    nxt += 1
                for f in fillers:
                    f()

            def epilogue_copies(blk):
                pvs = epi.tile([H + 1, TQ], BF16, tag=f"pvs{blk}")
                for j in range(NDIAG):
                    c = bass.ts(j, P)
                    nc.vector.tensor_copy(pvs[:, c], pvs_ps[blk][0:H + 1, c])
                return pvs

            def epi_tr_step(blk, pvs, j):
                def go():
                    ot = pstr.tile([P, H + 1], BF16, tag="tr")
                    nc.tensor.transpose(ot[:], pvs[:, bass.ts(j, P)],
                                        identb[0:H + 1, 0:H + 1])
                    r = epi.tile([P, 1], F32, tag="r")
                    nc.vector.reciprocal(r[:], ot[:, H:H + 1])
                    ob = bass.ts(blk * NDIAG + j, H)
                    obt = epi.tile([P, H], BF16, tag="ob")
                    nc.vector.tensor_scalar_mul(obt[:], ot[:, 0:H], r[:])
                    nc.sync.dma_start(out=out_d[:, ob], in_=obt[:])
                return go

            # ---- master schedule ----
            diag = lambda blk: [(blk, "d", c, c == 0, False) for c in range(NDIAG)]
            S1 = proj_xq_steps(1)
            S2 = proj_xk_steps(0)
            S3 = proj_xk_steps(1)
            S4 = proj_xk_steps(2)

            for step in proj_xq_steps(0):
                step()
            pv0 = pspv.tile([H + 1, TQ], F32, tag="pv")
            pvs_ps[0] = pv0
            attn_phase(diag(0), [(d, vdh, d * P) for d in range(NDIAG)],
                       fillers=S1[:4])
            for step in S1[4:]:
                step()
            pv1 = pspv.tile([H + 1, TQ], F32, tag="pv")
            pvs_ps[1] = pv1
            attn_phase(diag(1), [(NDIAG + d, vdh, TQ + d * P) for d in range(NDIAG)],
                       fillers=S2)

            phA = ([(0, "f", c, False, c == 3) for c in range(4)] +
                   [(1, "f", c, False, False) for c in range(4)])
            attn_phase(phA, [(NBLK * NDIAG + c, vfu, c * P) for c in range(4)],
                       fillers=S3)
            pvs0 = epilogue_copies(0)

            attn_phase([(1, "f", c, False, False) for c in range(4, 8)],
                       [(NBLK * NDIAG + c, vfu, c * P) for c in range(4, 8)],
                       fillers=S4)

            attn_phase([(1, "f", c, False, c == 11) for c in range(8, 12)],
                       [(NBLK * NDIAG + c, vfu, c * P) for c in range(8, 12)],
                       fillers=[epi_tr_step(0, pvs0, j) for j in range(NDIAG)])

            pvs1 = epilogue_copies(1)
            for j in range(NDIAG):
                epi_tr_step(1, pvs1, j)()
    nc.compile()
    return nc


def _pack_x(xT, cols):
    # xT: [C, T] fp32 -> [P, CCH*W] bf16 in SBUF layout
    a = xT[:, cols]                                   # [C, W]
    a = a.reshape(CCH, P, -1).transpose(1, 0, 2)      # [P, CCH, W]
    return np.ascontiguousarray(a.reshape(P, -1)).astype(NPBF)


def _pack_w(w):
    # w: [C, width] -> [P, CCH*width]
    a = w.reshape(CCH, P, -1).transpose(1, 0, 2)
    return np.ascontiguousarray(a.reshape(P, -1)).astype(NPBF)


def _host_inputs(x, Wk, Wq, Wv):
    wqv = _pack_w(np.concatenate([Wq, Wv], axis=1))
    wkv = _pack_w(np.concatenate([Wk, Wv], axis=1))
    wk = _pack_w(Wk)
    ii = np.arange(P)
    stair = (np.arange(896)[None, :] >= ii[:, None] + 384).astype(NPBF)
    identb = np.eye(P, dtype=NPBF)
    in_maps = []
    for b in range(B):
        xT = np.ascontiguousarray(x[b].T.astype(np.float32))  # [C, T]
        for h in range(2):
            q0s = (0, 1024) if h == 0 else (512, 1536)
            xq = np.stack([_pack_x(xT, slice(q0, q0 + TQ)) for q0 in q0s])
            xk = np.stack([_pack_x(xT, slice(i * TQ, (i + 1) * TQ))
                           for i in range(NKCH)])
            # ktb bias rows: row blk, col t = 0 if chunk t//128 is a (strictly
            # pre-diagonal) causal chunk for this core's block blk, else BIGNEG
            kaug = np.full((2, KFULL), BIGNEG, np.float32)
            for blk, q0 in enumerate(q0s):
                kaug[blk, :q0] = 0.0
            aug = kaug.astype(NPBF)
            in_maps.append(dict(xq=xq, xk=xk, wqv=wqv, wk=wk, wkv=wkv,
                                aug=aug, stair=stair, identb=identb))
    return in_maps


def kernel(x, Wk, Wq, Wv, trace=False):
    x = np.asarray(x, np.float32)
    in_maps = _host_inputs(x, np.asarray(Wk, np.float32),
                           np.asarray(Wq, np.float32), np.asarray(Wv, np.float32))
    if "nc" not in _CACHE:
        _CACHE["nc"] = build()
    nc = _CACHE["nc"]
    res = run_bass_kernel_spmd(nc, in_maps, list(range(8)), trace=trace)
    out = np.empty((B, T, H), np.float32)
    for b in range(B):
        for h in range(2):
            o = res.results[b * 2 + h]["out"]  # [P, NBLK*NDIAG*H] bf16
            o = np.asarray(o).astype(np.float32).reshape(P, NBLK, NDIAG, H)
            q0s = (0, 1024) if h == 0 else (512, 1536)
            for blk, q0 in enumerate(q0s):
                # row q0 + j*128 + p  <-  o[p, blk, j, :]
                out[b, q0:q0 + TQ] = o[:, blk].transpose(1, 0, 2).reshape(TQ, H)
    kernel.last_exec_time_ns = res.exec_time_ns
    kernel.last_results = res
    return out
